# revision 62
# baseline (speedup 1.0000x reference)
"""Trainium2 Bass kernel for nn_EncoderBlock (T5-style encoder block with the
torch flat `view(B*H, S, dh)` attention semantics — no head transpose).

Sharding: 8 cores = 4 batches x 2 sequence halves; attention is fully local
to each 64-token slab under the flat view, so there is no cross-core traffic.

Device-side design (HW-measured ~370 us/core vs 427 us for the prior
baseline; trace-driven):
  - Every large matmul runs in bf16 (PE: 1 cyc/row vs 4 for fp32), fp32
    accumulation in PSUM. Weights are cast + pre-tiled on the host so each
    DMA reads >= 2 KB contiguous per partition line. x ships bf16 (QK/V
    input) AND fp32 (residual), both host-pre-transposed [D, SQ].
  - ALL activation functions (exp, ln, square, relu, identity) come from
    the single natural_log_exp_and_others ACT table set — one table load
    for the whole kernel (bacc's greedy chooser is overridden at compile).
  - Softmax: aw = exp(bias)*exp(scores) with exp(bias) host-precomputed;
    1/Z = exp(-ln Z) on ACT (the DVE's iterative-divide reciprocal costs
    8 cyc/elem and serialized the attention phase). LN rstd likewise uses
    exp(-0.5*ln(var+eps)) — no sqrt-table swap, no DVE reciprocal.
  - HAM (PE clock gate) management: a dummy accumulating-matmul chain
    pre-warms the PE during the DMA lead-in, V-projection interleaves into
    the ACT(exp)-bound attention loop as real PE filler, AV matmuls lag
    scores by 4 cc-steps (the in-order PE stream never stalls on the
    exp->mult chain), and a small zero-matmul filler tops up PE duty so
    the 1.2->2.4 GHz gate stays open through attention.
  - FFN1 is decoupled from LN1: it consumes RAW y (bf16) with a rank-1
    (-colsum1 x mean) fixup accumulated into its PSUM, and rstd applied at
    eviction (relu is positively homogeneous) — the PE never waits for the
    LN chain. h1 normalization for the FFN2 residual happens lazily under
    the FFN1 matmuls. g1/be1 fold into W1'/b1'/b2' on the host.
  - DMA: W1 panels prefetch into a dedicated persistent SBUF region (the
    scoped pool's space overlaps attention tiles, which blocked its DMAs
    behind pool release); W1/W2/em/out traffic is spread across the
    sync/scalar/gpsimd queues.
  - fp8 (DoubleRow) for the FFN was tried and REJECTED: relative error of
    a random-sign dot product does not shrink with K, so each fp8 matmul
    costs ~5% on its output -> 2.7e-2 norm-rel, over the 2e-2 gate.
"""

import math
import sys
import time

import numpy as np

sys.path.insert(0, "/opt/trn_rl_repo")

import ml_dtypes  # noqa: E402

import concourse.bass as bass  # noqa: E402
import concourse.tile as tile  # noqa: E402
from concourse import bacc, mybir  # noqa: E402
from concourse.bass_utils import run_bass_kernel_spmd  # noqa: E402

B, S, D, H, F = 4, 1024, 1024, 16, 4096
DH = D // H  # 64
P = 128
SQ = S // 2  # per-core query rows (512)
ND = D // P  # 8 d-chunks
NF = F // P  # 32 f-chunks
NB = 8  # blocks (slabs) per core
NUM_BUCKETS, MAX_DISTANCE = 32, 128
LN_EPS = 1e-5
F32 = mybir.dt.float32
BF16 = mybir.dt.bfloat16
AF = mybir.ActivationFunctionType
OP = mybir.AluOpType

_CACHE = {}

# schedule knobs (sweepable)
KNOBS = dict(
    w1=8, w1pre=4, psf=4, ep=7, em=13, zs=1, pso=4, pss=2, wu=50, lag=4, fill=3
)


def _bucket_np(rel):
    """numpy replica of reference._relative_position_bucket (fp32 faithful)."""
    n = -rel
    num_buckets = NUM_BUCKETS // 2  # 16
    ret = (n < 0).astype(np.int32) * num_buckets
    n = np.abs(n)
    max_exact = num_buckets // 2  # 8
    is_small = n < max_exact
    val_if_large = max_exact + (
        np.log(np.maximum(n, 1).astype(np.float32) / max_exact)
        / np.float32(math.log(MAX_DISTANCE / max_exact))
        * (num_buckets - max_exact)
    ).astype(np.int32)
    val_if_large = np.minimum(val_if_large, num_buckets - 1)
    return ret + np.where(is_small, n, val_if_large)


def _build_em2(rel_bias):
    """em2[hg, c~, a~] = exp(bias) in bf16, both axes g-major permuted."""
    r = np.arange(-1023, 1024)
    v = rel_bias[_bucket_np(r)].astype(np.float32)  # (2047, H)
    idx = np.arange(1024)
    g, sl = idx // 64, idx % 64
    vidx = 16 * (sl[None, :] - sl[:, None]) + (g[None, :] - g[:, None]) + 1023
    em2 = np.empty((H, 1024, 1024), dtype=ml_dtypes.bfloat16)
    for hg in range(H):
        em2[hg] = np.exp(v[vidx, hg]).astype(ml_dtypes.bfloat16)
    return em2


def _declare_io(nc):
    def din(name, shape, dt=F32):
        return nc.dram_tensor(name, shape, dt, kind="ExternalInput").ap()

    a = {
        "xT_q": din("xT_q", (D, SQ)),
        "xb_q": din("xb_q", (D, SQ), BF16),
        # pre-tiled bf16 weights: [p, outer, inner, cols]
        "wqt": din("wqt", (P, ND, ND, P), BF16),
        "wkt": din("wkt", (P, ND, ND, P), BF16),
        "wvt": din("wvt", (P, ND, D), BF16),
        "wot": din("wot", (P, ND, ND, P), BF16),
        "w1t": din("w1t", (P, NF, ND, P), BF16),  # rows pre-scaled by g1
        "w2t": din("w2t", (P, ND, NF, P), BF16),
        # packed per-partition constant columns:
        # [0:8]=bo [8:16]=g1 [16:24]=g2 [24:32]=be2 [32:64]=b1'
        "cvec": din("cvec", (P, 64)),
        # packed bf16 bias rows: [0]=bq [1]=bk [2]=bv [3]=b2+be1
        "brows": din("brows", (4, D), BF16),
        # -colsum1[f] = -sum_d W1'[d, f] for the FFN1 rank-1 mean fixup
        "c1row": din("c1row", (1, F), BF16),
        "maskm": din("maskm", (P, NB), BF16),
        "em2": din("em2", (NB, 1024, 1024), BF16),
    }
    out = nc.dram_tensor("out", (D, SQ), F32, kind="ExternalOutput").ap()
    return a, out


def _single_act_table(arch):
    """Every ACT func this kernel uses (exp, ln, square, relu, identity,
    copy) lives in the natural_log_exp_and_others set. bacc's greedy
    table chooser would thrash exp_and_others <-> natural_log sets on
    every attention block (~1.3us per swap); emptying the other sets
    (order preserved, so act_func_set_id indices stay valid) forces one
    load for the whole kernel."""
    import concourse.hw_specs as hw_specs

    tables = hw_specs.get_activation_tables(arch)
    keep = "natural_log_exp_and_others"
    assert keep in tables
    return {n: (fns if n == keep else set()) for n, fns in tables.items()}


def _build_nc(debug=False, nrep=1):
    nc = bacc.Bacc("TRN2", target_bir_lowering=False, debug=debug, num_devices=8)
    a, out = _declare_io(nc)
    with tile.TileContext(nc) as tc:
        with nc.allow_low_precision(
            reason="bf16 matmuls with fp32 PSUM accumulation; tolerance is "
            "2e-2 norm-rel and bf16 keeps us under ~1e-2"
        ):
            for _ in range(nrep):
                _emit(nc, tc, a, out)
    import concourse.bacc as bacc_mod

    forced = _single_act_table(nc.m.arch)
    orig = bacc_mod.get_activation_tables
    bacc_mod.get_activation_tables = lambda arch: forced
    try:
        nc.compile()
    finally:
        bacc_mod.get_activation_tables = orig
    return nc


def _emit(nc, tc, a, out):
    fp = F32

    # persistent activation pools (strict LIFO nesting)
    cst_cm = tc.tile_pool(name="cst", bufs=1)
    cst = cst_cm.__enter__()
    pool_h1_cm = tc.tile_pool(name="p_h1", bufs=1)
    pool_h1 = pool_h1_cm.__enter__()
    pool_w1s_cm = tc.tile_pool(name="p_w1s", bufs=1)
    pool_w1s = pool_w1s_cm.__enter__()
    pool_xtq_cm = tc.tile_pool(name="p_xtq", bufs=1)
    pool_xtq = pool_xtq_cm.__enter__()
    pool_attT_cm = tc.tile_pool(name="p_attT", bufs=1)
    pool_attT = pool_attT_cm.__enter__()
    pool_wo_cm = tc.tile_pool(name="p_wo", bufs=1)
    pool_wo = pool_wo_cm.__enter__()
    pool_att_in_cm = tc.tile_pool(name="p_att_in", bufs=1)
    pool_att_in = pool_att_in_cm.__enter__()

    xTq = [pool_xtq.tile([P, SQ], fp, tag=f"xTq{d}", name=f"xTq{d}") for d in range(ND)]
    # xTb is dead after phase C (V-proj) — lives in the att_in pool so its
    # SBUF frees before the FFN phase
    xTb = [
        pool_att_in.tile([P, SQ], BF16, tag=f"xTb{d}", name=f"xTb{d}")
        for d in range(ND)
    ]

    ones_col = cst.tile([P, 1], BF16, tag="ones_col", name="ones_col")
    nc.vector.memset(ones_col, 1.0)
    ones_row = cst.tile([1, 512], BF16, tag="ones_row", name="ones_row")
    nc.vector.memset(ones_row, 1.0)
    eps_t = cst.tile([1, 1], fp, tag="eps_t", name="eps_t")
    nc.vector.memset(eps_t, LN_EPS)
    # preload ACT function tables while the prologue is DMA-bound. Every
    # ACT func below lives in the natural_log_exp_and_others set (exp, ln,
    # relu, square, identity, copy) so the whole kernel needs ONE table
    # load — sqrt/reciprocal sets are never touched.
    warm = cst.tile([1, 1], fp, tag="act_warm", name="act_warm")
    for af in (AF.Exp, AF.Ln, AF.Square, AF.Relu, AF.Identity):
        nc.scalar.activation(warm, eps_t, af)
    # PE pre-warm: a dense dummy-matmul stream during the DMA lead-in
    # flips the HAM clock gate to 8/8 (~3.4us of sustained PE activity)
    # so the real QK matmuls start at 2.4 GHz instead of 1.2
    pewu = cst.tile([P, P], BF16, tag="pe_warm", name="pe_warm")
    nc.vector.memset(pewu, 0.0)
    with tc.tile_pool(name="ps_wu", bufs=1, space="PSUM") as ps_wu:
        ps_w = ps_wu.tile([P, P], F32, tag="ps_wu", name="ps_wu")
        for i in range(KNOBS["wu"]):
            # accumulating chain (zeros) — back-to-back streaming, no
            # per-MM WAW drain stall
            nc.tensor.matmul(
                ps_w, pewu, pewu, start=(i == 0), stop=(i == KNOBS["wu"] - 1)
            )
    cvec = cst.tile([P, 64], fp, tag="cvec", name="cvec")
    bo_sb, g1_sb, g2_sb, be2_sb = (
        cvec[:, 0:8], cvec[:, 8:16], cvec[:, 16:24], cvec[:, 24:32])
    b1_sb = cvec[:, 32:64]
    def brow_sb(i, name):
        t = cst.tile([1, D], BF16, tag=f"brow{name}", name=f"brow{name}")
        nc.sync.dma_start(t[:], a["brows"][i : i + 1, :])
        return t


    # QPT/KPT[,j,:]: partitions [0:64] = block 2j, [64:128] = block 2j+1;
    # free = a~/c~ = g*64 + sl (g-major pseudo order)
    QPT = pool_att_in.tile([P, NB // 2, 1024], BF16, tag="QPT", name="QPT")
    KPT = pool_att_in.tile([P, NB // 2, 1024], BF16, tag="KPT", name="KPT")
    # vext[hl][pp, cc, 0:64] = pseudo-natural V chunk cc; [.., 64:128] = mask
    # replicated 64x so the AV matmul emits Z broadcast over 64 partitions
    vext = [
        pool_att_in.tile([P, 8, P], BF16, tag=f"vext{k}", name=f"vext{k}")
        for k in range(NB)
    ]
    # attT[, d, :]: att^T d-chunk (partitions = two 64-feature groups)
    attT = pool_attT.tile([P, ND, SQ], BF16, tag="attT", name="attT")
    wot = [
        pool_wo.tile([P, ND, P], BF16, tag=f"wot{e}", name=f"wot{e}")
        for e in range(ND)
    ]

    # ------- phase A+B: x load interleaved with Q/K projections -------
    with (
        tc.tile_pool(name="wqk", bufs=4) as wqk,
        tc.tile_pool(name="ps_b", bufs=4, space="PSUM") as ps_b,
    ):
        # DMA queue order: first Q panel, then x in bf16 (host pre-cast —
        # half the bytes, no DVE casts) — PE starts as soon as x lands.
        wp0 = wqk.tile([P, ND, P], BF16, tag="wqk", name="wqk")
        nc.sync.dma_start(wp0[:], a["wqt"][:, 0, :, :])
        for d in range(ND):
            # alternate DMA queues: ACT is idle this early, so its queue
            # carries half the x stream and the rings overlap on HW
            eng = nc.sync if d % 2 == 0 else nc.scalar
            eng.dma_start(xTb[d][:], a["xb_q"][d * P : (d + 1) * P, :])
        bqr = brow_sb(0, "q")
        bkr = brow_sb(1, "k")
        # the fp32 x (residual input, first read in phase D) loads behind
        # the QK weights on the scalar queue
        for d in range(ND):
            nc.scalar.dma_start(xTq[d][:], a["xT_q"][d * P : (d + 1) * P, :])
        for wname, brow, dst in (("wqt", bqr, QPT), ("wkt", bkr, KPT)):
            for di in range(ND):
                if wname == "wqt" and di == 0:
                    wp = wp0
                else:
                    wp = wqk.tile([P, ND, P], BF16, tag="wqk", name="wqk")
                    nc.sync.dma_start(wp[:], a[wname][:, di, :, :])
                # free axis = token hl*64+sl decomposed [j:4, parity:2, sl:64]
                ps = ps_b.tile([P, 4, 2, 64], F32, tag="ps_b", name="ps_b")
                for dj in range(ND):
                    nc.tensor.matmul(
                        ps, wp[:, dj, :], xTb[dj], start=(dj == 0), stop=False
                    )
                nc.tensor.matmul(
                    ps,
                    brow[:, di * P : (di + 1) * P],
                    ones_row,
                    start=False,
                    stop=True,
                )
                # batched pseudo-layout eviction: 4 blocks per DVE op
                for par in range(2):
                    g = 2 * di + par
                    for parity in range(2):
                        nc.vector.tensor_copy(
                            dst[
                                parity * 64 : parity * 64 + 64,
                                0:4,
                                g * 64 : g * 64 + 64,
                            ],
                            ps[par * 64 : par * 64 + 64, :, parity, :],
                        )

    # ------- phase C: V projection interleaved with attention -------
    bvr = brow_sb(2, "v")
    b2er = brow_sb(3, "e")
    c1r = cst.tile([1, F], BF16, tag="c1r", name="c1r")
    nc.sync.dma_start(c1r[:], a["c1row"][:, :])
    nc.sync.dma_start(cvec[:], a["cvec"][:, :])
    # prefetch the first W1 panels into a dedicated persistent region —
    # the regular w1 pool overlaps attention-phase SBUF, so its DMAs
    # can't start until the attention pools release (a ~50us DMA hole)
    w1pre = []
    for fi in range(KNOBS["w1pre"]):
        t = pool_w1s.tile([P, ND, P], BF16, tag=f"w1s{fi}", name=f"w1s{fi}")
        nc.scalar.dma_start(t[:], a["w1t"][:, fi, :, :])
        w1pre.append(t)
    maskm = cst.tile([P, NB], BF16, tag="maskm", name="maskm")
    nc.sync.dma_start(maskm[:], a["maskm"][:, :])
    for hl in range(NB):
        nc.vector.tensor_copy(
            vext[hl][:, :, 64:128],
            maskm[:, :, None].broadcast_to([P, NB, 64]),
        )
    # V projection interleaved with attention: the V matmuls are the real
    # PE filler that keeps duty above the HAM warm threshold during the
    # ACT(exp)-bound attention stretch; a small dummy-MM chain tops it up.
    with (
        tc.tile_pool(name="wvp", bufs=1) as wvp,
        tc.tile_pool(name="expp", bufs=KNOBS["ep"]) as epool,
        tc.tile_pool(name="emp", bufs=KNOBS["em"]) as empool,
        tc.tile_pool(name="zsb", bufs=KNOBS["zs"]) as zpool,
        tc.tile_pool(name="ps_v", bufs=1, space="PSUM") as ps_v,
        tc.tile_pool(name="ps_keep", bufs=1, space="PSUM") as ps_keep,
        tc.tile_pool(name="ps_s", bufs=KNOBS["pss"], space="PSUM") as ps_s,
        tc.tile_pool(name="ps_av", bufs=1, space="PSUM") as ps_av,
    ):
        pans = []
        for dj in range(ND):
            wp = wvp.tile([P, D], BF16, tag=f"wvp{dj}", name=f"wvp{dj}")
            nc.sync.dma_start(wp[:], a["wvt"][:, dj, :])
            pans.append(wp)
        pek = ps_keep.tile([P, 512], F32, tag="pe_keep", name="pe_keep")
        nfill = 0

        def _fill(n):
            # HAM warm-keeper: accumulating zero-matmuls on a dedicated bank
            nonlocal nfill
            for _ in range(n):
                nc.tensor.matmul(
                    pek[:, 0:P], pewu, pewu, start=(nfill == 0), stop=False
                )
                nfill += 1

        for tk in range(SQ // P):
            for half in range(2):
                # free axis = feat g*64+r decomposed [j:4, parity:2, r:64]
                ps = ps_v.tile([P, 4, 2, 64], F32, tag="ps_v", name="ps_v")
                for dj in range(ND):
                    nc.tensor.matmul(
                        ps,
                        xTb[dj][:, tk * P : (tk + 1) * P],
                        pans[dj][:, half * 512 : half * 512 + 512],
                        start=(dj == 0),
                        stop=False,
                    )
                nc.tensor.matmul(
                    ps,
                    ones_row[:, :P],
                    bvr[:, half * 512 : half * 512 + 512],
                    start=False,
                    stop=True,
                )
                for sp in range(2):
                    hl = 2 * tk + sp
                    for parity in range(2):
                        nc.vector.tensor_copy(
                            vext[hl][
                                parity * 64 : parity * 64 + 64,
                                half * 4 : half * 4 + 4,
                                0:64,
                            ],
                            ps[sp * 64 : sp * 64 + 64, :, parity, :],
                        )
            for sp in range(2):
                hl = 2 * tk + sp
                ems = []
                for cc in range(8):
                    em = empool.tile([P, 1024], BF16, tag="emp", name="emp")
                    eng = nc.sync if cc % 2 == 0 else nc.gpsimd
                    eng.dma_start(em[:], a["em2"][hl, cc * P : (cc + 1) * P, :])
                    ems.append(em)
                # keep the sync queue clear for em tables
                nc.scalar.dma_start(wot[hl][:], a["wot"][:, hl, :, :])
                jb, ro = hl // 2, (hl % 2) * 64
                # pav: free = [hv:2, j:4, parity:2, sl:64] (both query
                # halves in one 2-bank tile -> single-op 1/Z chain)
                # partitions 0:64 att features, 64:128 Z replicated
                pav = ps_av.tile([P, 2, 4, 2, 64], F32, tag="ps_av", name="ps_av")
                aws = []

                def _av(cc):
                    for hv in range(2):
                        nc.tensor.matmul(
                            pav[:, hv, :, :, :],
                            vext[hl][:, cc, :],
                            aws[cc][:, hv * 512 : hv * 512 + 512],
                            start=(cc == 0),
                            stop=(cc == 7),
                        )

                # AV lags scores by LAG cc-steps so exp+mult for AV(cc)
                # are done when the in-order PE stream reaches it
                LAG = KNOBS["lag"]
                for cc in range(8):
                    # 2-bank score tile: both query halves side by side
                    ps = ps_s.tile([P, 1024], F32, tag="ps_s", name="ps_s")
                    for hv in range(2):
                        nc.tensor.matmul(
                            ps[:, hv * 512 : hv * 512 + 512],
                            KPT[ro : ro + 64, jb, cc * P : (cc + 1) * P],
                            QPT[ro : ro + 64, jb, hv * 512 : hv * 512 + 512],
                            start=True,
                            stop=True,
                        )
                    ex = epool.tile([P, 1024], BF16, tag="expp", name="expp")
                    nc.scalar.activation(ex, ps, AF.Exp)
                    # aw = em * exp(s), in place over the exp tile
                    nc.vector.tensor_tensor(ex, ems[cc], ex, OP.mult)
                    aws.append(ex)
                    _fill(KNOBS["fill"])
                    if cc >= LAG:
                        _av(cc - LAG)
                for cc in range(8 - LAG, 8):
                    _av(cc)
                # 1/Z = exp(-ln Z) on ACT (same table set as Exp — no DVE
                # iterative-divide, no table swap); one op covers both hv
                lnz = zpool.tile([DH, 2, 4, 2, 64], fp, tag="lnz", name="lnz")
                nc.scalar.activation(lnz, pav[DH : 2 * DH, :, :, :, :], AF.Ln)
                zb = zpool.tile([DH, 2, 4, 2, 64], BF16, tag="zb", name="zb")
                nc.scalar.activation(zb, lnz, AF.Exp, scale=-1.0)
                for hv in range(2):
                    # batched attT eviction: 4 g-groups per op
                    for parity in range(2):
                        nc.vector.tensor_tensor(
                            attT[
                                parity * 64 : parity * 64 + 64,
                                hv * 4 : hv * 4 + 4,
                                hl * 64 : hl * 64 + 64,
                            ],
                            pav[0:64, hv, :, parity, :],
                            zb[:, hv, :, parity, :],
                            OP.mult,
                        )
        if nfill:
            nc.tensor.matmul(pek[:, 0:P], pewu, pewu, start=False, stop=True)

    pool_att_in_cm.__exit__(None, None, None)

    # --- phase D: O-projection + residual + LN1 stats, interleaved ---
    # yb = raw y = (att@Wo + bo + x) in bf16. FFN1 consumes yb DIRECTLY
    # (no normalize barrier): ps = W1'^T y - colsum1*mu (rank-1 fixup),
    # then *rstd at eviction — PE never waits for the LN1 chain.
    h1b = [
        pool_h1.tile([P, SQ], BF16, tag=f"h1b_{d}", name=f"h1b_{d}")
        for d in range(ND)
    ]
    yb = [
        pool_h1.tile([P, SQ], BF16, tag=f"yb_{d}", name=f"yb_{d}")
        for d in range(ND)
    ]
    with (
        tc.tile_pool(name="lnt", bufs=3) as lnt,
        tc.tile_pool(name="ps_o", bufs=KNOBS["pso"], space="PSUM") as ps_o,
        tc.tile_pool(name="ps_st", bufs=1, space="PSUM") as ps_st,
    ):
        ps_u = ps_st.tile([1, 512], F32, tag="ps_u", name="ps_u")
        ps_q = ps_st.tile([1, 512], F32, tag="ps_q", name="ps_q")
        for ei in range(ND):
            ps = ps_o.tile([P, 512], F32, tag="ps_o", name="ps_o")
            for di in range(ND):
                nc.tensor.matmul(
                    ps,
                    wot[ei][:, di, :],
                    attT[:, di, :],
                    start=(di == 0),
                    stop=(di == ND - 1),
                )
            # y = (o + bo) + x, bias via per-partition STT operand
            nc.vector.scalar_tensor_tensor(
                xTq[ei], ps, bo_sb[:, ei : ei + 1], xTq[ei], OP.add, OP.add
            )
            # LN1 stats, interleaved
            nc.vector.tensor_copy(yb[ei], xTq[ei])
            sq = lnt.tile([P, SQ], BF16, tag="ln_sq", name="ln_sq")
            nc.scalar.activation(sq, xTq[ei], AF.Square)
            nc.tensor.matmul(
                ps_u, ones_col, yb[ei], start=(ei == 0), stop=(ei == ND - 1)
            )
            nc.tensor.matmul(ps_q, ones_col, sq, start=(ei == 0), stop=(ei == ND - 1))

        # chain OUTPUTS live in the persistent pool_h1 (needed through
        # phase E); temps stay in the scoped lnt pool
        mean_b, rstd_b, mu_row = _ln_chain(
            nc, lnt, pool_h1, ps_st, ps_u, ps_q, ones_row, eps_t, "1"
        )

    pool_wo_cm.__exit__(None, None, None)
    pool_attT_cm.__exit__(None, None, None)

    # ------- phase E: FFN + residual + LN2 stats interleaved + store -------
    y2 = [pool_h1.tile([P, SQ], fp, tag=f"y2_{d}", name=f"y2_{d}") for d in range(ND)]
    with (
        tc.tile_pool(name="ff1", bufs=1) as ffpool,
        tc.tile_pool(name="w1p", bufs=KNOBS["w1"]) as w1pool,
        tc.tile_pool(name="w2p", bufs=2) as w2pool,
        tc.tile_pool(name="lnt2", bufs=2) as lnt2,
        tc.tile_pool(name="lns2", bufs=1) as lns2,
        tc.tile_pool(name="onat", bufs=3) as opool,
        tc.tile_pool(name="ps_f", bufs=KNOBS["psf"], space="PSUM") as ps_f,
        tc.tile_pool(name="ps_st2", bufs=1, space="PSUM") as ps_st2,
    ):
        ff1 = [
            ffpool.tile([P, SQ], BF16, tag=f"ff1_{i}", name=f"ff1_{i}")
            for i in range(NF)
        ]
        for fi in range(NF):
            if fi < KNOBS["w1pre"]:
                w1p = w1pre[fi]
            else:
                w1p = w1pool.tile([P, ND, P], BF16, tag="w1p", name="w1p")
                eng = nc.sync if fi % 2 == 0 else nc.scalar
                eng.dma_start(w1p[:], a["w1t"][:, fi, :, :])
            ps = ps_f.tile([P, 512], F32, tag="ps_f", name="ps_f")
            for dj in range(ND):
                nc.tensor.matmul(
                    ps, w1p[:, dj, :], yb[dj], start=(dj == 0), stop=False
                )
            # rank-1 mean fixup: ps += (-colsum1)[f] * mu[q]
            nc.tensor.matmul(
                ps,
                c1r[:, fi * P : (fi + 1) * P],
                mu_row,
                start=False,
                stop=True,
            )
            m = lnt2.tile([P, SQ], fp, tag="ffm", name="ffm")
            nc.vector.tensor_tensor(m, ps, rstd_b, OP.mult)
            nc.scalar.activation(ff1[fi], m, AF.Relu, bias=b1_sb[:, fi : fi + 1])
        # lazy h1 normalize (for the FFN2 residual), overlapped with FFN1 MMs
        for ei in range(ND):
            t = lnt2.tile([P, SQ], fp, tag="ln_tmp", name="ln_t")
            nc.gpsimd.tensor_tensor(t, xTq[ei], mean_b, OP.subtract)
            nc.vector.tensor_tensor(h1b[ei], t, rstd_b, OP.mult)
        ps_u2 = ps_st2.tile([1, 512], F32, tag="ps_u2", name="ps_u2")
        ps_q2 = ps_st2.tile([1, 512], F32, tag="ps_q2", name="ps_q2")
        for ei in range(ND):
            w2p = w2pool.tile([P, NF, P], BF16, tag="w2p", name="w2p")
            eng = nc.sync if ei % 2 == 0 else nc.scalar
            eng.dma_start(w2p[:], a["w2t"][:, ei, :, :])
            ps = ps_f.tile([P, 512], F32, tag="ps_f", name="ps_f")
            for fj in range(NF):
                nc.tensor.matmul(
                    ps,
                    w2p[:, fj, :],
                    ff1[fj],
                    start=(fj == 0),
                    stop=False,
                )
            # + (b2 + be1) via rank-1 bf16 matmul
            nc.tensor.matmul(
                ps, b2er[:, ei * P : (ei + 1) * P], ones_row, start=False, stop=True
            )
            # y2 = g1*h1_nog + (W2 ff + b2 + be1)
            nc.vector.scalar_tensor_tensor(
                y2[ei], h1b[ei], g1_sb[:, ei : ei + 1], ps, OP.mult, OP.add
            )
            # LN2 stats, interleaved
            yb2 = lnt2.tile([P, SQ], BF16, tag="ln_yb2", name="ln_yb2")
            nc.gpsimd.tensor_copy(yb2, y2[ei])
            sq = lnt2.tile([P, SQ], BF16, tag="ln_sq2", name="ln_sq2")
            nc.scalar.activation(sq, y2[ei], AF.Square)
            nc.tensor.matmul(ps_u2, ones_col, yb2, start=(ei == 0), stop=(ei == ND - 1))
            nc.tensor.matmul(ps_q2, ones_col, sq, start=(ei == 0), stop=(ei == ND - 1))
        mean_b2, rstd_b2, _ = _ln_chain(
            nc, lnt2, lns2, ps_st2, ps_u2, ps_q2, ones_row, eps_t, "2"
        )
        for ei in range(ND):
            t = lnt2.tile([P, SQ], fp, tag="ln_tmp2", name="ln_t2")
            nc.gpsimd.tensor_tensor(t, y2[ei], mean_b2, OP.subtract)
            t2 = lnt2.tile([P, SQ], fp, tag="ln_tm2", name="ln_tm2")
            nc.vector.tensor_tensor(t2, t, rstd_b2, OP.mult)
            o = opool.tile([P, SQ], fp, tag="onat", name=f"onat{ei}")
            nc.scalar.activation(
                o, t2, AF.Identity,
                bias=be2_sb[:, ei : ei + 1], scale=g2_sb[:, ei : ei + 1],
            )
            # spread the 2MB output store across three DMA queues
            seng = (nc.sync, nc.scalar, nc.gpsimd)[ei % 3]
            seng.dma_start(out[ei * P : (ei + 1) * P, :], o[:])

    pool_xtq_cm.__exit__(None, None, None)
    pool_w1s_cm.__exit__(None, None, None)
    pool_h1_cm.__exit__(None, None, None)
    cst_cm.__exit__(None, None, None)


def _ln_chain(nc, tmp, outp, ps_st, ps_u, ps_q, ones_row, eps_t, sfx):
    """mean/rstd from accumulated sum/sumsq PSUM rows. rstd via ACT
    exp(-0.5*ln(var+eps)) — stays in the natural_log_exp table set (no
    sqrt-set swap, no DVE iterative divide). Returns broadcast [P, SQ]
    fp32 (mean_b, rstd_b) allocated in `outp`, plus the bf16 [1, SQ]
    mean row (for the FFN1 rank-1 fixup)."""
    fp = F32
    mean = tmp.tile([1, SQ], fp, tag="st_mean", name="st_mean")
    nc.vector.tensor_scalar_mul(mean, ps_u, 1.0 / D)
    meanb = outp.tile([1, SQ], BF16, tag=f"st_meanb{sfx}", name=f"st_meanb{sfx}")
    nc.vector.tensor_copy(meanb, mean)
    ps_m = ps_st.tile([P, 512], F32, tag="ps_m", name="ps_m")
    nc.tensor.matmul(ps_m, ones_row[:, :P], meanb, start=True, stop=True)
    mean_b = outp.tile([P, SQ], fp, tag=f"mean_b{sfx}", name=f"mean_b{sfx}")
    nc.vector.tensor_copy(mean_b, ps_m)
    msq = tmp.tile([1, SQ], fp, tag="st_msq", name="st_msq")
    nc.vector.tensor_tensor(msq, mean, mean, OP.mult)
    var = tmp.tile([1, SQ], fp, tag="st_var", name="st_var")
    nc.vector.scalar_tensor_tensor(var, ps_q, 1.0 / D, msq, OP.mult, OP.subtract)
    lnv = tmp.tile([1, SQ], fp, tag="st_lnv", name="st_lnv")
    nc.scalar.activation(lnv, var, AF.Ln, bias=eps_t)
    rstdb = tmp.tile([1, SQ], BF16, tag="st_rstdb", name="st_rstdb")
    nc.scalar.activation(rstdb, lnv, AF.Exp, scale=-0.5)
    ps_r = ps_st.tile([P, 512], F32, tag="ps_r", name="ps_r")
    nc.tensor.matmul(ps_r, ones_row[:, :P], rstdb, start=True, stop=True)
    rstd_b = outp.tile([P, SQ], fp, tag=f"rstd_b{sfx}", name=f"rstd_b{sfx}")
    nc.vector.tensor_copy(rstd_b, ps_r)
    return mean_b, rstd_b, meanb


def _prep_in_maps(inputs):
    bf = ml_dtypes.bfloat16

    def f32(k):
        return np.ascontiguousarray(np.asarray(inputs[k], dtype=np.float32))

    x = f32("in_state")
    mask = np.asarray(inputs["padding_mask"]).astype(np.float32)
    em2_full = _build_em2(np.asarray(inputs["rel_bias"], dtype=np.float32))
    idx = np.arange(1024)
    perm_idx = (idx % 64) * 16 + idx // 64  # c~ -> true pseudo index

    def tile4(w, n_out, n_in):
        # w: (K, M) fp32 -> (P, n_out, n_in, P) bf16 with
        # [p, o, i, m] = w[i*P + p, o*P + m]
        K, M = w.shape
        assert K == n_in * P and M == n_out * P
        t = w.reshape(n_in, P, n_out, P).transpose(1, 2, 0, 3)
        return np.ascontiguousarray(t.astype(bf))

    w1 = f32("W1")
    be1 = f32("ln1_b")
    g1 = f32("ln1_g")
    w1s = g1[:, None] * w1  # fold LN1 gain into W1 rows
    b1p = f32("b1") + be1 @ w1  # fold LN1 bias into b1
    b2e = f32("b2") + be1  # fold LN1 bias into the FFN2 output bias
    c1row = (-w1s.sum(axis=0))[None, :]  # rank-1 mean-fixup coefficients
    cvec = np.zeros((P, 64), np.float32)
    cvec[:, 0:8] = f32("bo").reshape(8, P).T
    cvec[:, 8:16] = g1.reshape(8, P).T
    cvec[:, 16:24] = f32("ln2_g").reshape(8, P).T
    cvec[:, 24:32] = f32("ln2_b").reshape(8, P).T
    cvec[:, 32:64] = b1p.reshape(32, P).T
    brows = np.stack([
        np.asarray(inputs["bq"], np.float32),
        np.asarray(inputs["bk"], np.float32),
        np.asarray(inputs["bv"], np.float32),
        b2e,
    ]).astype(bf)

    shared = {
        "wqt": tile4(f32("Wq"), ND, ND),
        "wkt": tile4(f32("Wk"), ND, ND),
        "wvt": np.ascontiguousarray(
            f32("Wv").reshape(ND, P, D).transpose(1, 0, 2).astype(bf)
        ),
        "wot": tile4(f32("Wo"), ND, ND),
        "w1t": tile4(w1s, NF, ND),
        "w2t": tile4(f32("W2"), ND, NF),
        "cvec": cvec,
        "brows": brows,
        "c1row": np.ascontiguousarray(c1row.astype(bf)),
    }
    in_maps = []
    for c in range(8):
        b, half = c // 2, c % 2
        q0 = half * SQ
        m = dict(shared)
        m["xT_q"] = np.ascontiguousarray(x[b, q0 : q0 + SQ, :].T)
        m["xb_q"] = np.ascontiguousarray(m["xT_q"].astype(bf))
        m["maskm"] = np.ascontiguousarray(
            mask[b][perm_idx].reshape(NB, P).T.astype(bf))
        m["em2"] = np.ascontiguousarray(em2_full[half * NB : half * NB + NB])
        in_maps.append(m)
    return in_maps


def kernel(**inputs) -> np.ndarray:
    if "nc" not in _CACHE:
        _CACHE["nc"] = _build_nc()
    nc = _CACHE["nc"]
    in_maps = _prep_in_maps(inputs)
    t0 = time.perf_counter()
    res = run_bass_kernel_spmd(nc, in_maps, core_ids=list(range(8)))
    _CACHE["last_run_s"] = time.perf_counter() - t0
    out = np.empty((B, S, D), dtype=np.float32)
    for c in range(8):
        b, half = c // 2, c % 2
        out[b, half * SQ : half * SQ + SQ, :] = res.results[c]["out"].T
    return out

